# revision 5
# baseline (speedup 1.0000x reference)
"""Trainium2 Bass kernel for capsule dynamic routing (nn_Capsule) — v2.

Reference (per batch item b):
    u = x_b @ W; logits = 0
    for i in 4:
        c = softmax(logits, axis=capsule)
        t_j = sum_s c[s,j] * u[s, j*64:(j+1)*64]; v = squash(t)
        if i < 3: logits[s,j] += u[s, jblk] . v_j

Never materializes u (linearity):
    y_j   = sum_s c[s,j] x_s            y-GEMM   (c stationary, col-tiled)
    t     = W^T y^T                     t-GEMM   (w16 stationary per-slice)
    P^T   = Vblk^T W^T                  P-GEMM   (vblk stationary, block-diag)
    upd^T = P^T X                       upd-GEMM (P slices stationary, col-tiled)

v2 structural changes vs v1 (335 us):
  - dense b*16+j packing for t/P GEMMs: N=512/N=128-dense streams, far fewer
    matmul instructions than v1's 32-padded N=256 forms.
  - all transposes via HWDGE dma_start_transpose on sync/scalar queues
    (zero PE transposes; PE does only real GEMM work, HAM stays warm).
  - rsqrt in squash as exp(-0.5*ln(s+eps)): ln+exp+softmax-exp all live in
    the natural_log_exp_and_others ACT table set -> one table load total
    (v1 thrashed sqrt<->exp sets: 8 loads + drains).
  - input loads split across all 3 DMA queues: gpsimd casts f32->f16
    directly; sync/scalar load raw f32 quarters into staging, DVE/ACT cast.
  - per-iteration DMA transposes (y^T, P, upd^T) run on otherwise-idle
    HWDGE queues, overlapped with PE.

HW lessons kept from v1:
  - DVE copy PSUM(f32) -> SBUF(f16) kills the device; every PSUM->f16 cast
    goes through ScalarE activation(Copy).
  - matmul start=True lazily zeroes the whole 2KB PSUM bank: accumulation
    groups must own a (partition-range x bank) region exclusively;
    partition-disjoint groups interleave with skip_group_check=True.
    Non-accumulating MMs into disjoint columns of one bank are fine after
    the first start=True (has_written is per-element).
  - nc.vector.memset on f16 tiles is unreliable: load constants from host.
"""
import numpy as np
from contextlib import ExitStack

import concourse.bass as bass
import concourse.bacc as bacc
import concourse.tile as tile
from concourse import mybir
from concourse.bass_utils import run_bass_kernel_spmd

f16 = mybir.dt.float16
f32 = mybir.dt.float32
COPY = mybir.ActivationFunctionType.Copy
EXP = mybir.ActivationFunctionType.Exp
LN = mybir.ActivationFunctionType.Ln

S, B, H = 512, 64, 1024
NCAP, DCAP = 16, 64
ROUTINGS = 4
N_CORES = 8
BL = B // N_CORES          # 8 batch items per core
SC = S // 128              # 4 s-chunks
HC = H // 128              # 8 h-chunks
OC = H // 128              # 8 o-chunks (o = NCAP*DCAP = 1024)


def _act_copy(nc, out, in_):
    nc.scalar.activation(out=out, in_=in_, func=COPY, scale=1.0, alpha=0.0)


def _build_kernel(tc, out_d, x_d, w_d, c0_d, logits_d, vblk_d, ones2_d,
                  o2t_d, eps_d, zeros_d):
    nc = tc.nc
    ctx = ExitStack()
    const = ctx.enter_context(tc.tile_pool(name="const", bufs=1))
    ld = ctx.enter_context(tc.tile_pool(name="ld", bufs=2))
    work = ctx.enter_context(tc.tile_pool(name="work", bufs=1))
    small = ctx.enter_context(tc.tile_pool(name="small", bufs=2))
    ps_big = ctx.enter_context(tc.tile_pool(name="ps_big", bufs=2,
                                            space="PSUM"))
    ps_u = ctx.enter_context(tc.tile_pool(name="ps_u", bufs=2, space="PSUM"))
    ps_sm = ctx.enter_context(tc.tile_pool(name="ps_sm", bufs=2,
                                           space="PSUM"))

    # ---------- persistent tensors ----------
    x16 = const.tile([128, BL, SC, 1024], f16)    # (s_loc, b, sc, h)
    # xt: per-b transposes write the contiguous (m=sc*8+hc, r) enumeration
    xt16 = const.tile([128, BL, SC, HC, 128], f16)  # (h_loc, b, sc, hc, s_loc)
    w16 = const.tile([128, HC, 1024], f16)        # (h_loc, hc, o)
    wt16 = const.tile([128, OC, 1024], f16)       # (o_loc, oc, h)
    c16 = const.tile([128, SC, BL, 32], f16)      # coupling coeffs, 16-31 zero
    logits = const.tile([128, SC, BL, NCAP], f32)
    vblk = const.tile([128, OC, 128], f16)        # block-diag v, bj dense
    ones2 = const.tile([128, 2], f16)             # [[1;0],[0;1]] halves
    o2t = const.tile([2, 128], f32)               # broadcast helper
    eps = const.tile([2, 1], f32)
    zeros = const.tile([128, 128], f16)           # zero-weight for psum init

    # ---------- loads ----------
    # gpsimd (SWDGE, casts): consts + x batches 0-1
    nc.gpsimd.dma_start(out=c16[:], in_=c0_d[:])
    nc.gpsimd.dma_start(out=logits[:], in_=logits_d[:])
    nc.gpsimd.dma_start(out=vblk[:], in_=vblk_d[:])
    nc.gpsimd.dma_start(out=ones2[:], in_=ones2_d[:])
    nc.gpsimd.dma_start(out=o2t[:], in_=o2t_d[:])
    nc.gpsimd.dma_start(out=eps[:], in_=eps_d[:])
    nc.gpsimd.dma_start(out=zeros[:], in_=zeros_d[:])
    xr = x_d.rearrange("(sc p) b h -> p b sc h", p=128)
    wr = w_d.rearrange("(hc p) o -> p hc o", p=128)
    for b in (0, 1):
        nc.gpsimd.dma_start(out=x16[:, b, :, :], in_=xr[:, b, :, :])
    # sync/scalar (HWDGE, no cast): raw f32 quarter-batches + engine casts.
    # W first (t-GEMM needs it right after y completes).
    i = 0
    for hc in range(HC):
        q = nc.sync if i % 2 == 0 else nc.scalar
        st = ld.tile([128, 1024], f32, tag=f"st{i % 2}", name=f"w{hc}")
        q.dma_start(out=st[:], in_=wr[:, hc, :])
        if i % 2 == 0:
            nc.vector.tensor_copy(w16[:, hc, :], st[:])
        else:
            _act_copy(nc, w16[:, hc, :], st[:])
        i += 1
    for b in (2, 3, 4, 5, 6, 7):
        for sc in range(SC):
            q = nc.sync if i % 2 == 0 else nc.scalar
            st = ld.tile([128, 1024], f32, tag=f"st{i % 2}",
                         name=f"x{b}_{sc}")
            q.dma_start(out=st[:], in_=xr[:, b, sc, :])
            if i % 2 == 0:
                nc.vector.tensor_copy(x16[:, b, sc, :], st[:])
            else:
                _act_copy(nc, x16[:, b, sc, :], st[:])
            i += 1
    # W^T: 8 transposes from w16 (needed by P-GEMM, mid-iteration 0)
    for hc in range(HC):
        q = nc.sync if hc % 2 == 0 else nc.scalar
        q.dma_start_transpose(wt16[:, :, hc * 128:(hc + 1) * 128],
                              w16[:, hc, :])
    # X^T: one transpose per batch (1MB each), needed at upd of iteration 0
    for b in range(BL):
        q = nc.sync if b % 2 == 0 else nc.scalar
        q.dma_start_transpose(xt16[:, b], x16[:, b])

    v32 = None
    for it in range(ROUTINGS):
        last = it == ROUTINGS - 1

        # ---------- y = C^T X: (16 rows per 32-strip x 4 batches) ------
        y_ps = [ps_big.tile([128, 1024], f32, tag="big", name=f"y{it}_{g}")
                for g in range(2)]
        for g in range(2):
            for b_ in range(4):
                b = 4 * g + b_
                for sc in range(SC):
                    for half in range(2):
                        nc.tensor.matmul(
                            y_ps[g][32 * b_:32 * b_ + 32,
                                    512 * half:512 * half + 512],
                            c16[:, sc, b, :],
                            x16[:, b, sc, 512 * half:512 * half + 512],
                            start=(sc == 0), stop=(sc == SC - 1),
                            skip_group_check=True,
                            tile_position=(0, 32 * b_))
        # evac f32->f16 (ScalarE) then DMA-transpose -> yt (padded cols)
        y_sb = work.tile([128, 2, 1024], f16, tag="y_sb")
        yt = work.tile([128, HC, 256], f16, tag="yt")
        for g in range(2):
            _act_copy(nc, y_sb[:, g, :], y_ps[g][:])
            q = nc.sync if g == 0 else nc.scalar
            q.dma_start_transpose(yt[:, :, 128 * g:128 * g + 128],
                                  y_sb[:, g, :])
        # dense rhs view: cols (g, b_, j) == b*16+j order, skipping j-pad
        yt_dense = yt.rearrange("p hc (g b_ jp) -> p hc g b_ jp", g=2, jp=32)

        # ---------- t = W^T y^T: out (o, bj dense) ----------
        t_ps = ps_big.tile([128, 1024], f32, tag="big", name=f"t{it}")
        for oc in range(OC):
            for hc in range(HC):
                nc.tensor.matmul(
                    t_ps[:, oc * 128:oc * 128 + 128],
                    w16[:, hc, oc * 128:oc * 128 + 128],
                    yt_dense[:, hc, :, :, 0:16],
                    start=(hc == 0 and oc % 4 == 0), stop=(hc == HC - 1),
                    skip_group_check=True)

        # ---------- extract block-diag: t_sb (e*64+d, b*8+oc) ----------
        t_sb = small.tile([128, 64], f32, tag="t_sb")
        tv = t_ps.rearrange("p (oc b j) -> p oc b j", oc=OC, b=BL)
        dv = t_sb.rearrange("p (b oc) -> p b oc", oc=OC)
        for oc in range(OC):
            nc.vector.tensor_copy(dv[0:64, :, oc], tv[0:64, oc, :, 2 * oc])
            nc.vector.tensor_copy(dv[64:128, :, oc],
                                  tv[64:128, oc, :, 2 * oc + 1])

        # ---------- squash: v = t * exp(-0.5*ln(sum_d t^2 + eps)) -------
        t2 = small.tile([128, 64], f16, tag="t2")
        nc.vector.tensor_mul(t2[:], t_sb[:], t_sb[:])
        sq_ps = ps_sm.tile([2, 512], f32, tag="sm", name=f"sq{it}")
        nc.tensor.matmul(sq_ps[:, 0:64], ones2[:], t2[:])
        ln_sb = small.tile([2, 64], f32, tag="ln_sb")
        nc.scalar.activation(out=ln_sb[:], in_=sq_ps[:, 0:64], func=LN,
                             bias=eps[:], scale=1.0, alpha=0.0)
        rs = small.tile([2, 64], f32, tag="rs")
        nc.scalar.activation(out=rs[:], in_=ln_sb[:], func=EXP,
                             scale=-0.5, alpha=0.0)
        bc_ps = ps_sm.tile([128, 512], f32, tag="sm", name=f"bc{it}")
        nc.tensor.matmul(bc_ps[:, 0:64], o2t[:], rs[:])
        bc_sb = small.tile([128, 64], f32, tag="bc_sb")
        nc.vector.tensor_copy(bc_sb[:], bc_ps[:, 0:64])
        if last:
            v32 = small.tile([128, 64], f32, tag="v32")
            nc.vector.tensor_mul(v32[:], t_sb[:], bc_sb[:])
            break
        v16 = small.tile([128, 64], f16, tag="v16")
        nc.vector.tensor_mul(v16[:], t_sb[:], bc_sb[:])

        # ---------- scatter v into block-diag vblk ----------
        # vblk[e*64+d, oc, b*16+2oc+e] = v16[e*64+d, b*8+oc]
        sv = v16.rearrange("p (b o) -> p b o", o=OC)
        for oc in range(OC):
            for e in range(2):
                dst = vblk[64 * e:64 * e + 64, oc, :].rearrange(
                    "p (b r) -> p b r", r=16)[:, :, 2 * oc + e]
                nc.vector.tensor_copy(dst, sv[64 * e:64 * e + 64, :, oc])

        # ---------- P^T = Vblk^T W^T: out (bj dense, h) ----------
        pT_ps = ps_big.tile([128, 1024], f32, tag="big", name=f"pT{it}")
        for oc in range(OC):
            for half in range(2):
                nc.tensor.matmul(
                    pT_ps[:, 512 * half:512 * half + 512],
                    vblk[:, oc, :],
                    wt16[:, oc, 512 * half:512 * half + 512],
                    start=(oc == 0), stop=(oc == OC - 1),
                    skip_group_check=True)
        pT_sb = work.tile([128, 1024], f16, tag="pT_sb")
        _act_copy(nc, pT_sb[:, 0:512], pT_ps[:, 0:512])
        _act_copy(nc, pT_sb[:, 512:1024], pT_ps[:, 512:1024])
        p_sb = work.tile([128, HC, 128], f16, tag="p_sb")
        nc.sync.dma_start_transpose(p_sb[:], pT_sb[:])

        # ---------- upd^T = P^T X (col-tiled per batch) ----------
        u_ps = [ps_u.tile([128, 512], f32, tag="u", name=f"u{it}_{g}")
                for g in range(2)]
        for g in range(2):
            # zero-fill the whole tile (pad rows incl.) so the f16 evac
            # below reads fully-defined psum; real MMs accumulate onto 0.
            nc.tensor.matmul(u_ps[g][:], zeros[:], x16[:, 0, 0, 0:512],
                             start=True, stop=False, skip_group_check=True)
            for b_ in range(4):
                b = 4 * g + b_
                for hc in range(HC):
                    nc.tensor.matmul(
                        u_ps[g][32 * b_:32 * b_ + 16, :],
                        p_sb[:, hc, 16 * b:16 * b + 16],
                        xt16[:, b, :, hc, :],
                        start=False, stop=(hc == HC - 1),
                        skip_group_check=True,
                        tile_position=(0, 32 * b_))
        # evac f32->f16, DMA-transpose to (s_loc, sc, bjp), add to logits
        u_sb = work.tile([128, 2, 512], f16, tag="u_sb")
        ut = work.tile([128, 2, SC, 128], f16, tag="ut")
        for g in range(2):
            _act_copy(nc, u_sb[:, g, :], u_ps[g][:])
            q = nc.sync if g == 0 else nc.scalar
            q.dma_start_transpose(ut[:, g], u_sb[:, g, :])
        for g in range(2):
            src = ut[:, g].rearrange("p sc (b_ jp) -> p sc b_ jp", jp=32)
            nc.vector.tensor_add(
                logits[:, :, 4 * g:4 * g + 4, :],
                logits[:, :, 4 * g:4 * g + 4, :], src[:, :, :, 0:16])

        # ---------- softmax over capsules -> c16 ----------
        for g in range(2):
            ex = small.tile([128, SC, 4, NCAP], f32, tag="ex")
            nc.scalar.activation(out=ex[:],
                                 in_=logits[:, :, 4 * g:4 * g + 4, :],
                                 func=EXP, scale=1.0, alpha=0.0)
            sm = small.tile([128, SC, 4, 1], f32, tag="sm")
            nc.vector.reduce_sum(sm[:], ex[:], axis=mybir.AxisListType.X)
            rc = small.tile([128, SC, 4, 1], f32, tag="rc")
            nc.vector.reciprocal(rc[:], sm[:])
            nc.vector.tensor_mul(c16[:, :, 4 * g:4 * g + 4, 0:16], ex[:],
                                 rc.broadcast_to([128, SC, 4, NCAP]))

    # ---------- out[b, 2*oc+e, d] = v32[e*64+d, b*8+oc] ----------
    out_ap = bass.AP(tensor=out_d.tensor, offset=0,
                     ap=[[1, 128], [1024, BL], [128, 8]])
    nc.sync.dma_start(out=out_ap, in_=v32.rearrange("p (b o) -> p b o", o=8))
    ctx.close()


_CACHE = {}


def _host_consts():
    c0 = np.zeros((128, SC, BL, 32), np.float32)
    c0[:, :, :, 0:NCAP] = 1.0 / NCAP
    logi = np.zeros((128, SC, BL, NCAP), np.float32)
    vblk0 = np.zeros((128, OC, 128), np.float32)
    ones2 = np.zeros((128, 2), np.float32)
    ones2[0:64, 0] = 1.0
    ones2[64:128, 1] = 1.0
    o2t = np.zeros((2, 128), np.float32)
    o2t[0, 0:64] = 1.0
    o2t[1, 64:128] = 1.0
    eps = np.full((2, 1), 1e-7, np.float32)
    zeros = np.zeros((128, 128), np.float32)
    return {"c0i": c0, "logi": logi, "vblki": vblk0, "ones2": ones2,
            "o2t": o2t, "eps": eps, "zeros": zeros}


def _get_nc():
    if "nc" not in _CACHE:
        nc = bacc.Bacc("TRN2", target_bir_lowering=False, debug=False)
        x_d = nc.dram_tensor("x", [S, BL, H], f32, kind="ExternalInput")
        w_d = nc.dram_tensor("w", [H, NCAP * DCAP], f32, kind="ExternalInput")
        c0_d = nc.dram_tensor("c0i", [128, SC, BL, 32], f32,
                              kind="ExternalInput")
        logits_d = nc.dram_tensor("logi", [128, SC, BL, NCAP], f32,
                                  kind="ExternalInput")
        vblk_d = nc.dram_tensor("vblki", [128, OC, 128], f32,
                                kind="ExternalInput")
        ones2_d = nc.dram_tensor("ones2", [128, 2], f32, kind="ExternalInput")
        o2t_d = nc.dram_tensor("o2t", [2, 128], f32, kind="ExternalInput")
        eps_d = nc.dram_tensor("eps", [2, 1], f32, kind="ExternalInput")
        zeros_d = nc.dram_tensor("zeros", [128, 128], f32,
                                 kind="ExternalInput")
        out_d = nc.dram_tensor("out", [BL, NCAP, DCAP], f32,
                               kind="ExternalOutput")
        with tile.TileContext(nc) as tc:
            _build_kernel(tc, out_d.ap(), x_d.ap(), w_d.ap(), c0_d.ap(),
                          logits_d.ap(), vblk_d.ap(), ones2_d.ap(),
                          o2t_d.ap(), eps_d.ap(), zeros_d.ap())
        nc.compile()
        _CACHE["nc"] = nc
    return _CACHE["nc"]


def kernel(inputs: np.ndarray, W: np.ndarray, _trace: bool = False):
    """inputs: (512, 64, 1024) f32; W: (1, 1024, 1024) f32.
    Returns (64, 16, 64) f32."""
    nc = _get_nc()
    consts = _host_consts()
    wf = np.ascontiguousarray(W[0].astype(np.float32))
    in_maps = []
    for c in range(N_CORES):
        m = {"x": np.ascontiguousarray(
                 inputs[:, c * BL:(c + 1) * BL, :].astype(np.float32)),
             "w": wf}
        m.update(consts)
        in_maps.append(m)
    kw = {}
    if _trace:
        kw = dict(trace=True, trace_cores=[0], stitch_traces=False)
    res = run_bass_kernel_spmd(nc, in_maps, core_ids=list(range(N_CORES)),
                               **kw)
    out = np.concatenate([res.results[c]["out"] for c in range(N_CORES)],
                         axis=0)
    if _trace:
        return out.astype(np.float32), res
    return out.astype(np.float32)
